# revision 25
# baseline (speedup 1.0000x reference)
"""GAT attention kernel for Trainium2 (Bass/Tile), 8-core data parallel.

Per-core math (2 examples each, N=256 items, D=64):
  e   = LayerNorm(emb);  ua = e[0] * e[2:]
  qk  = LeakyReLU(s_q_i + s_k_j + c);  alpha = softmax_j
  attention over value_ij = LN(ua_i * ua_j) collapsed via gram matrices:
    mu = UA@UA^T/D,  E2 = UA^2@UA^2^T/D,  isig = exp(-0.5*ln(E2-mu^2+eps))
    att_i = g*((ua_i*St_i - ct_i) * rden_i) + b
  with St = beta~@UA, beta~ = exp(qk)*isig (unnormalized), rden = 1/sum_j exp,
  ct_i = rowsum(ua_i*St_i)/D  (uses mu_ij = ua_i.ua_j/D).
  out = LeakyReLU(concat([e0*e1], att))

Perf structure (v2):
  - all big matmuls in bf16 (grams, transposes, S, denominator LDWEIGHTS);
    score broadcasts in fp32r so qk stays fp32 into exp.
  - isig and LN rstds via Exp(-0.5*Ln(x)): every ACT func used (square,
    parametric_relu, identity, exp, ln, copy) lives in one act table ->
    exactly ONE ACT_TABLE_LOAD, no mid-kernel switch, no exp/rsqrt barrier.
  - idle GpSimd engine takes psum->sbuf copies, msq, and scalar fixups.
  - PE queue ordered: both examples' score pipelines before the grams so
    the exps are in flight while grams run.
"""

import numpy as np

import concourse.bass as bass
from concourse import bacc
import concourse.mybir as mybir
import concourse.tile as tile
from concourse import masks
from concourse.bass_utils import run_bass_kernel_spmd

F32 = mybir.dt.float32
F32R = mybir.dt.float32r
BF16 = mybir.dt.bfloat16
ALU = mybir.AluOpType
ACTF = mybir.ActivationFunctionType
AX = mybir.AxisListType

B, NODE, D = 16, 258, 64
N = NODE - 2
N_CORES = 8
B_LOC = B // N_CORES
EPS = 1e-5
SLOPE = 0.01
OUT_ROWS = N + 1


def build():
    nc = bacc.Bacc()
    emb = nc.dram_tensor("emb", [B_LOC, NODE, D], F32, kind="ExternalInput")
    cstT = nc.dram_tensor("cstT", [D, 2], F32, kind="ExternalInput")   # cols: vq, vk
    cstR = nc.dram_tensor("cstR", [1, 4 * D], F32, kind="ExternalInput")  # [g|b|vi|C0..]
    out = nc.dram_tensor("out", [B_LOC, OUT_ROWS, D], F32, kind="ExternalOutput")

    with tile.TileContext(nc) as tc:
        with (
            tc.tile_pool(name="const", bufs=1) as cpool,
            tc.tile_pool(name="work", bufs=2) as pool,
            tc.tile_pool(name="psmall", bufs=2, space="PSUM") as psmall,
            tc.tile_pool(name="pmid", bufs=2, space="PSUM") as pmid,
            tc.tile_pool(name="pqk", bufs=2, space="PSUM") as pqk,
            tc.tile_pool(name="pgram", bufs=2, space="PSUM") as pgram,
        ):
            # ---- input DMAs first on the sync queue: they gate everything ----
            # U rows of both examples land in ONE tile at quadrant partitions
            # {0,32,64,96}: uid_e0, iid_e0, uid_e1, iid_e1.
            tU4 = cpool.tile([128, D], F32)
            nc.gpsimd.memset(tU4[:], 0.0)
            u4v = tU4[:].rearrange("(a b) d -> a b d", b=32)
            nc.sync.dma_start(u4v[0:2, 0:1, :], emb[0, 0:2, :])
            nc.sync.dma_start(u4v[2:4, 0:1, :], emb[1, 0:2, :])
            # item rows 2..257 as [128, 2, 64], row r = 2p + n
            in_tiles = []
            for e in range(B_LOC):
                tAB = pool.tile([128, 2, D], F32, tag=f"tAB{e}")
                nc.sync.dma_start(tAB[:], emb[e, 2:258, :].rearrange("(p n) d -> p n d", n=2))
                in_tiles.append(tAB)

            cst_sb = cpool.tile([1, 4 * D], F32)
            nc.gpsimd.dma_start(cst_sb[:], cstR[:, :])
            g_row = cst_sb[:, 0:D]
            b_row = cst_sb[:, D:2 * D]
            vi_row = cst_sb[:, 2 * D:3 * D]
            c0_sb = cst_sb[:, 3 * D:3 * D + 1]
            vqk = cpool.tile([D, 2], F32)
            nc.gpsimd.dma_start(vqk[:], cstT[:, :])

            # ---- global constants ----
            identF = cpool.tile([128, 128], F32)
            masks.make_identity(nc, identF[:])
            identR = cpool.tile([128, 128], F32R)
            nc.scalar.copy(identR[:], identF[:])
            ones1f = cpool.tile([1, 128], F32)
            nc.vector.memset(ones1f[:], 1.0)
            ones1 = cpool.tile([1, 128], F32R)
            nc.scalar.copy(ones1[:], ones1f[:])
            ones2f = cpool.tile([128, 2], F32)
            nc.vector.memset(ones2f[:], 1.0)
            ones2b = cpool.tile([128, 2], BF16)
            nc.scalar.copy(ones2b[:], ones2f[:])
            eps_col = cpool.tile([128, 1], F32)
            nc.gpsimd.memset(eps_col[:], EPS)

            vqk_bf = cpool.tile([D, 2], BF16)
            nc.scalar.copy(vqk_bf[:], vqk[:])
            gb_rowr = cpool.tile([1, 2 * D], F32R)
            nc.scalar.copy(gb_rowr[:], cst_sb[:, 0:2 * D])

            p_gb = psmall.tile([128, 2 * D], F32, tag="small")
            nc.tensor.matmul(p_gb[:], ones1[:], gb_rowr[:])
            gb_bc = cpool.tile([128, 2 * D], F32)
            nc.scalar.copy(gb_bc[:], p_gb[:])
            g_bc = gb_bc[:, 0:D]
            b_bc = gb_bc[:, D:2 * D]

            # ================= LN stats, batched [128, 5] ==================
            # col 0: U quadrant tile; cols 1+2e+n: example e slot n
            sum5 = pool.tile([128, 5], F32, tag="sum5")
            nc.vector.reduce_sum(sum5[:, 0:1], tU4[:], axis=AX.X)
            for e in range(B_LOC):
                nc.vector.reduce_sum(sum5[:, 1 + 2 * e:3 + 2 * e], in_tiles[e][:], axis=AX.X)
            nm5 = pool.tile([128, 5], F32, tag="nm5")
            nc.vector.tensor_scalar_mul(nm5[:], sum5[:], -1.0 / D)
            xc5 = pool.tile([128, 5, D], F32, tag="xc5")
            nc.vector.tensor_scalar_add(xc5[:, 0, :], tU4[:], nm5[:, 0:1])
            for e in range(B_LOC):
                for n in range(2):
                    k = 1 + 2 * e + n
                    nc.vector.tensor_scalar_add(xc5[:, k, :], in_tiles[e][:, n, :],
                                                nm5[:, k:k + 1])
            sq5 = pool.tile([128, 5, D], F32, tag="sq5")
            nc.vector.tensor_mul(sq5[:], xc5[:], xc5[:])
            ss5 = pool.tile([128, 5], F32, tag="ss5")
            nc.vector.reduce_sum(ss5[:], sq5[:], axis=AX.X)
            # rstd = exp(-0.5 * ln(ss/D + eps)); single act table (ln+exp).
            ln5 = pool.tile([128, 5], F32, tag="ln5")
            nc.scalar.activation(ln5[:], ss5[:], ACTF.Ln, bias=eps_col[:], scale=1.0 / D)
            rstd5 = pool.tile([128, 5], F32, tag="rstd5")
            nc.scalar.activation(rstd5[:], ln5[:], ACTF.Exp, scale=-0.5)

            # U rows LN (full tile; only quadrant rows are real)
            elnU4 = cpool.tile([128, D], F32)
            nc.vector.scalar_tensor_tensor(elnU4[:], xc5[:, 0, :], rstd5[:, 0:1],
                                           g_bc, op0=ALU.mult, op1=ALU.mult)
            nc.vector.tensor_add(elnU4[:], elnU4[:], b_bc)

            st = [dict() for _ in range(B_LOC)]

            # ============ per-example u0/iid rows + broadcast ==============
            for e in range(B_LOC):
                S = st[e]
                iid_row = pool.tile([1, D], F32, tag=f"iid{e}")
                nc.sync.dma_start(iid_row[:], elnU4[64 * e + 32:64 * e + 33, :])
                S["iid_row"] = iid_row
                if e == 0:
                    u0row = elnU4[0:1, :]
                else:
                    u0r = pool.tile([1, D], F32, tag="u0r")
                    nc.scalar.copy(u0r[:], elnU4[64:65, :])
                    u0row = u0r[:]
                S["u0row"] = u0row
                # [u0*g | u0*b] row, broadcast to all partitions via PE
                ugb_row = pool.tile([1, 2 * D], F32R, tag=f"ugb{e}")
                nc.vector.tensor_mul(ugb_row[:, 0:D], u0row, g_row)
                nc.vector.tensor_mul(ugb_row[:, D:2 * D], u0row, b_row)
                p_ugb = pmid.tile([128, 2 * D], F32, tag="mid")
                nc.tensor.matmul(p_ugb[:], ones1[:], ugb_row[:])
                S["p_ugb"] = p_ugb

                # c = vi . iid + c0, staged at partition 0 of a [1,1] tile
                si_scr = pool.tile([1, D], F32, tag="si_scr")
                nc.vector.tensor_mul(si_scr[:], iid_row[:], vi_row)
                si = pool.tile([1, 1], F32, tag="si")
                nc.vector.reduce_sum(si[:], si_scr[:], axis=AX.X)
                c_all = pool.tile([1, 1], F32, tag="c_all")
                nc.vector.tensor_scalar_add(c_all[:], si[:], c0_sb[:])
                S["c_all"] = c_all

            # ============ per-example ua (bf16) + transposes ===============
            for e in range(B_LOC):
                S = st[e]
                p_ugb = S["p_ugb"]
                ua_f = pool.tile([128, 2, D], F32R, tag=f"uaf{e}")
                for n in range(2):
                    k = 1 + 2 * e + n
                    t = pool.tile([128, D], F32, tag=f"uat{n}")
                    nc.vector.scalar_tensor_tensor(t[:], xc5[:, k, :], rstd5[:, k:k + 1],
                                                   p_ugb[:, 0:D], op0=ALU.mult, op1=ALU.mult)
                    nc.vector.tensor_add(ua_f[:, n, :], t[:], p_ugb[:, D:2 * D])
                ua_bf = pool.tile([128, 2, D], BF16, tag=f"ua{e}")
                nc.vector.tensor_copy(ua_bf[:], ua_f.bitcast(F32)[:])
                S["ua_f"] = ua_f
                S["ua_bf"] = ua_bf

                p_t = pmid.tile([D, N], F32R, tag="mid")
                nc.tensor.transpose(p_t[:, 0:128], ua_f[:, 0, :], identR[:])
                nc.tensor.transpose(p_t[:, 128:256], ua_f[:, 1, :], identR[:])
                uat = pool.tile([D, N], BF16, tag=f"uatT{e}")
                nc.vector.tensor_copy(uat[:], p_t.bitcast(F32)[:])
                ua2t = pool.tile([D, N], BF16, tag=f"ua2t{e}")
                nc.scalar.activation(ua2t[:], p_t.bitcast(F32)[:], ACTF.Square)
                S["uat"] = uat
                S["ua2t"] = ua2t

            # ============ per-example scores -> qk^T -> exp ================
            for e in range(B_LOC):
                S = st[e]
                uat = S["uat"]
                # s_k as columns (psum partition = j), s_q as a row; one psum
                # tile per example: cols 0:256 = s_q row, 256:258/258:260 = s_k
                skc = pool.tile([128, 2], F32, tag=f"skc{e}")
                p_sc = psmall.tile([128, N + 4], F32, tag="small")
                for J in range(2):
                    cs = slice(J * 128, (J + 1) * 128)
                    nc.tensor.matmul(p_sc[:, N + 2 * J:N + 2 * J + 2], uat[:, cs], vqk_bf[:])
                    nc.vector.tensor_copy(skc[:, J:J + 1], p_sc[:, N + 2 * J + 1:N + 2 * J + 2])
                nc.tensor.matmul(p_sc[0:1, 0:N], vqk_bf[:, 0:1], uat[:])
                sqc = pool.tile([1, N], F32R, tag=f"sqc{e}")
                nc.vector.tensor_scalar_add(sqc[:], p_sc[0:1, 0:N], S["c_all"][:])
                S["skc"] = skc
                S["sqc"] = sqc

            for e in range(B_LOC):
                S = st[e]
                p_sqbc = pqk.tile([128, N], F32, tag="qk")
                nc.tensor.matmul(p_sqbc[:], ones1[:], S["sqc"][:])
                expvTs = []
                for J in range(2):
                    qkT = pool.tile([128, N], F32, tag="qkT")
                    nc.scalar.activation(qkT[:], p_sqbc[:], ACTF.Prelu,
                                         bias=S["skc"][:, J:J + 1], alpha=SLOPE)
                    expvT = pool.tile([128, N], BF16, tag=f"expvT{e}{J}")
                    nc.scalar.activation(expvT[:], qkT[:], ACTF.Exp)
                    expvTs.append(expvT)
                S["expvTs"] = expvTs

            # ============ grams (PE) -> var -> isig ========================
            for e in range(B_LOC):
                S = st[e]
                uat = S["uat"]
                ua2t = S["ua2t"]
                msq = pool.tile([128, 2 * N], F32, tag=f"msq{e}")
                var = pool.tile([128, 2 * N], F32, tag=f"var{e}")
                for blk in range(2):
                    cs = slice(blk * 128, (blk + 1) * 128)
                    ns = slice(blk * N, (blk + 1) * N)
                    p_g = pgram.tile([128, 2, N], F32, tag="gram")
                    nc.tensor.matmul(p_g[:, 0, :], uat[:, cs], uat[:])
                    nc.tensor.matmul(p_g[:, 1, :], ua2t[:, cs], ua2t[:])
                    nc.scalar.activation(msq[:, ns], p_g[:, 0, :], ACTF.Square, scale=1.0 / D)
                    nc.vector.scalar_tensor_tensor(var[:, ns], p_g[:, 1, :], 1.0 / D,
                                                   msq[:, ns], op0=ALU.mult, op1=ALU.subtract)
                lnv = pool.tile([128, 2 * N], F32, tag=f"lnv{e}")
                nc.scalar.activation(lnv[:], var[:], ACTF.Ln, bias=eps_col[:])
                isig = pool.tile([128, 2 * N], BF16, tag=f"isig{e}")
                nc.scalar.activation(isig[:], lnv[:], ACTF.Exp, scale=-0.5)
                S["isig"] = isig

            # ============ denominators (needs exp, runs during grams) ======
            for e in range(B_LOC):
                S = st[e]
                expvTs = S["expvTs"]
                rden_cols = pool.tile([128, 2], F32, tag=f"rdenc{e}")
                p_den = psmall.tile([128, 4], F32, tag="small")
                for blk in range(2):
                    cs = slice(blk * 128, (blk + 1) * 128)
                    ds = slice(2 * blk, 2 * blk + 2)
                    nc.tensor.matmul(p_den[:, ds], expvTs[0][:, cs], ones2b[:],
                                     start=True, stop=False)
                    nc.tensor.matmul(p_den[:, ds], expvTs[1][:, cs], ones2b[:],
                                     start=False, stop=True)
                    nc.vector.reciprocal(rden_cols[:, blk:blk + 1], p_den[:, 2 * blk:2 * blk + 1])
                S["rden_cols"] = rden_cols

            # ================= pass B: attention + output ==================
            for e in range(B_LOC):
                S = st[e]
                ua_bf = S["ua_bf"]
                isig = S["isig"]

                btTs = []
                for J in range(2):
                    ns = slice(J * N, (J + 1) * N)
                    btT = pool.tile([128, N], BF16, tag=f"btT{J}")
                    nc.vector.tensor_mul(btT[:], S["expvTs"][J][:], isig[:, ns])
                    btTs.append(btT)

                p_S2 = pmid.tile([128, 2, D], F32, tag="mid")
                for blk in range(2):
                    cs = slice(blk * 128, (blk + 1) * 128)
                    nc.tensor.matmul(p_S2[:, blk, :], btTs[0][:, cs], ua_bf[:, 0, :],
                                     start=True, stop=False)
                    nc.tensor.matmul(p_S2[:, blk, :], btTs[1][:, cs], ua_bf[:, 1, :],
                                     start=False, stop=True)

                t1b = pool.tile([128, 2, D], F32, tag="t1b")
                ua_f = S["ua_f"]
                nc.vector.tensor_mul(t1b[:], ua_f.bitcast(F32)[:], p_S2[:])
                c_raw = pool.tile([128, 2], F32, tag="c_raw")
                nc.vector.reduce_sum(c_raw[:], t1b[:], axis=AX.X)
                c_col = pool.tile([128, 2], F32, tag="c_col")
                nc.vector.tensor_scalar_mul(c_col[:], c_raw[:], 1.0 / D)

                o_big = pool.tile([128, 2, D], F32, tag="o_big")
                for blk in range(2):
                    rg = pool.tile([128, D], F32, tag="rg")
                    nc.vector.tensor_scalar_mul(rg[:], g_bc, S["rden_cols"][:, blk:blk + 1])
                    t2 = pool.tile([128, D], F32, tag="t2")
                    nc.vector.scalar_tensor_tensor(t2[:], t1b[:, blk, :], c_col[:, blk:blk + 1],
                                                   rg[:], op0=ALU.subtract, op1=ALU.mult)
                    t3 = pool.tile([128, D], F32, tag="t3")
                    nc.vector.tensor_add(t3[:], t2[:], b_bc)
                    nc.vector.scalar_tensor_tensor(o_big[:, blk, :], t3[:], SLOPE, t3[:],
                                                   op0=ALU.mult, op1=ALU.max)
                out_rows = out[e, 1:257, :].rearrange("(p n) d -> p n d", n=2)
                (nc.sync if e == 0 else nc.gpsimd).dma_start(out_rows, o_big[:])

                ui = pool.tile([1, D], F32, tag="ui")
                nc.vector.tensor_mul(ui[:], S["u0row"], S["iid_row"][:])
                uo = pool.tile([1, D], F32, tag="uo")
                nc.vector.scalar_tensor_tensor(uo[:], ui[:], SLOPE, ui[:],
                                               op0=ALU.mult, op1=ALU.max)
                (nc.sync if e == 0 else nc.gpsimd).dma_start(out[e, 0:1, :], uo[:])

    nc.compile()
    return nc


def _host_consts(Wa, ba, a_w, a_b):
    aq, ak, ai = a_w[:D], a_w[D:2 * D], a_w[2 * D:]
    vq = aq @ Wa
    vk = ak @ Wa
    vi = ai @ Wa
    c0 = float(ba @ aq + ba @ ak + ba @ ai + a_b[0])
    cstT = np.stack([vq, vk], axis=1).astype(np.float32)
    cstR = np.zeros((1, 4 * D), np.float32)
    cstR[0, 2 * D:3 * D] = vi
    cstR[0, 3 * D] = c0
    return cstT, cstR


_NC_CACHE = {}


def _get_nc():
    if "nc" not in _NC_CACHE:
        _NC_CACHE["nc"] = build()
    return _NC_CACHE["nc"]


def run(embeddings, Wa, ba, a_w, a_b, ln_g, ln_b, **spmd_kwargs):
    embeddings = np.ascontiguousarray(embeddings, dtype=np.float32)
    cstT, cstR = _host_consts(np.asarray(Wa, np.float32), np.asarray(ba, np.float32),
                              np.asarray(a_w, np.float32), np.asarray(a_b, np.float32))
    cstR[0, 0:D] = np.asarray(ln_g, np.float32)
    cstR[0, D:2 * D] = np.asarray(ln_b, np.float32)

    nc = _get_nc()
    in_maps = [
        {"emb": embeddings[c * B_LOC:(c + 1) * B_LOC], "cstT": cstT, "cstR": cstR}
        for c in range(N_CORES)
    ]
    res = run_bass_kernel_spmd(nc, in_maps, core_ids=list(range(N_CORES)), **spmd_kwargs)
    outp = np.concatenate([res.results[c]["out"] for c in range(N_CORES)], axis=0)
    return outp, res


def kernel(embeddings, Wa, ba, a_w, a_b, ln_g, ln_b):
    outp, _ = run(embeddings, Wa, ba, a_w, a_b, ln_g, ln_b)
    return outp


# revision 28
# speedup vs baseline: 1.2571x; 1.2571x over previous
"""GAT attention kernel for Trainium2 (Bass/Tile), 8-core data parallel.

Per-core math (2 examples each, N=256 items, D=64):
  e   = LayerNorm(emb);  ua = e[0] * e[2:]
  qk  = LeakyReLU(s_q_i + s_k_j + c);  alpha = softmax_j
  attention over value_ij = LN(ua_i * ua_j) collapsed via gram matrices:
    mu = UA@UA^T/D,  E2 = UA^2@UA^2^T/D,  isig = rsqrt(E2-mu^2)
    att_i = g*((ua_i*St_i - ct_i) * rden_i) + b
  with St = beta~@UA, beta~ = exp(qk)*isig (unnormalized), rden = 1/sum_j exp,
  ct_i = rowsum(ua_i*St_i)/D  (uses mu_ij = ua_i.ua_j/D).
  out = LeakyReLU(concat([e0*e1], att))

Perf structure (v4):
  - ua, transposes, grams, scores, denominators, S-matmul all in bf16 on PE
    (1 cycle/row); qk built in fp32 psum so exp input stays fp32.
  - N^2 elementwise work merged into [128, 1024] ops across both examples:
    ONE Exp, ONE abs-rsqrt (isig), ONE btT mul, ONE msq stt, ONE var stt.
  - exactly 2 ACT table loads (exp set, then abs_rsqrt set), enforced by an
    explicit dep of isig on the merged exp.
  - msq/var on DVE so ACT goes exp -> isig directly; output rg/lrelu on ACT
    to drain the DVE tail; const casts on DVE in the idle startup window.
"""

import numpy as np

import concourse.bass as bass
from concourse import bacc
import concourse.mybir as mybir
import concourse.tile as tile
from concourse import masks
from concourse.bass_utils import run_bass_kernel_spmd
from concourse.tile import add_dep_helper

F32 = mybir.dt.float32
I32 = mybir.dt.int32
F32R = mybir.dt.float32r
BF16 = mybir.dt.bfloat16
ALU = mybir.AluOpType
ACTF = mybir.ActivationFunctionType
AX = mybir.AxisListType

B, NODE, D = 16, 258, 64
N = NODE - 2
N_CORES = 8
B_LOC = B // N_CORES
EPS = 1e-5
SLOPE = 0.01
OUT_ROWS = N + 1
MAGIC = 0x5f375a86


def _rsqrt(nc, pool, x, P, W, pfx):
    """x**-0.5 on DVE: bit trick + 1 Newton iteration. rel err ~1.8e-3."""
    y0 = pool.tile([P, W], F32, tag=pfx + "_y0")
    nc.vector.tensor_scalar(y0.bitcast(I32)[:], x.bitcast(I32)[:], 1, None,
                            op0=ALU.logical_shift_right)
    nc.vector.tensor_scalar(y0.bitcast(I32)[:], y0.bitcast(I32)[:], -1, MAGIC,
                            op0=ALU.mult, op1=ALU.add)
    t = pool.tile([P, W], F32, tag=pfx + "_t")
    nc.vector.tensor_mul(t[:], y0[:], y0[:])
    u = pool.tile([P, W], F32, tag=pfx + "_u")
    nc.vector.scalar_tensor_tensor(u[:], t[:], 0.5, x[:], op0=ALU.mult, op1=ALU.mult)
    v = pool.tile([P, W], F32, tag=pfx + "_v")
    nc.vector.tensor_mul(v[:], u[:], y0[:])
    r = pool.tile([P, W], F32, tag=pfx + "_r")
    nc.vector.scalar_tensor_tensor(r[:], y0[:], 1.5, v[:], op0=ALU.mult, op1=ALU.subtract)
    return r


def build():
    nc = bacc.Bacc()
    emb = nc.dram_tensor("emb", [B_LOC, NODE, D], F32, kind="ExternalInput")
    cstT = nc.dram_tensor("cstT", [D, 2], F32, kind="ExternalInput")   # cols: vq, vk
    cstR = nc.dram_tensor("cstR", [1, 4 * D], F32, kind="ExternalInput")  # [g|b|vi|C0..]
    out = nc.dram_tensor("out", [B_LOC, OUT_ROWS, D], F32, kind="ExternalOutput")

    with tile.TileContext(nc) as tc:
        with (
            tc.tile_pool(name="const", bufs=1) as cpool,
            tc.tile_pool(name="work", bufs=2) as pool,
            tc.tile_pool(name="psmall", bufs=1, space="PSUM") as psmall,
            tc.tile_pool(name="pmid", bufs=2, space="PSUM") as pmid,
            tc.tile_pool(name="pqk", bufs=1, space="PSUM") as pqk,
            tc.tile_pool(name="pgram", bufs=2, space="PSUM") as pgram,
        ):
            # ---- input DMAs first; two queues in parallel ----
            tU4 = cpool.tile([128, D], F32)
            nc.gpsimd.memset(tU4[:], 0.0)
            u4v = tU4[:].rearrange("(a b) d -> a b d", b=32)
            cst_sb = cpool.tile([1, 4 * D], F32)
            nc.gpsimd.dma_start(cst_sb[:], cstR[:, :])
            vqk = cpool.tile([D, 2], F32)
            nc.gpsimd.dma_start(vqk[:], cstT[:, :])

            nc.sync.dma_start(u4v[0:2, 0:1, :], emb[0, 0:2, :])
            nc.gpsimd.dma_start(u4v[2:4, 0:1, :], emb[1, 0:2, :])
            # item rows 2..257 as [128, 2, 64], row r = 2p + n
            in_tiles = []
            for e in range(B_LOC):
                tAB = pool.tile([128, 2, D], F32, tag=f"tAB{e}")
                (nc.sync if e == 0 else nc.gpsimd).dma_start(
                    tAB[:], emb[e, 2:258, :].rearrange("(p n) d -> p n d", n=2))
                in_tiles.append(tAB)

            g_row = cst_sb[:, 0:D]
            b_row = cst_sb[:, D:2 * D]
            vi_row = cst_sb[:, 2 * D:3 * D]
            c0_sb = cst_sb[:, 3 * D:3 * D + 1]

            # ---- global constants (casts on DVE; ACT stays free early) ----
            identF = cpool.tile([128, 128], F32)
            masks.make_identity(nc, identF[:])
            identB = cpool.tile([128, 128], BF16)
            nc.vector.tensor_copy(identB[:], identF[:])
            ones1f = cpool.tile([1, 128], F32)
            nc.vector.memset(ones1f[:], 1.0)
            ones1b = cpool.tile([1, 128], BF16)
            nc.vector.tensor_copy(ones1b[:], ones1f[:])
            ones2f = cpool.tile([128, 2], F32)
            nc.vector.memset(ones2f[:], 1.0)
            ones2b = cpool.tile([128, 2], BF16)
            nc.vector.tensor_copy(ones2b[:], ones2f[:])
            vqk_bf = cpool.tile([D, 2], BF16)
            nc.vector.tensor_copy(vqk_bf[:], vqk[:])
            gb_rowb = cpool.tile([1, 2 * D], BF16)
            nc.vector.tensor_copy(gb_rowb[:], cst_sb[:, 0:2 * D])

            p_gb = psmall.tile([128, 2 * D], F32, tag="small")
            nc.tensor.matmul(p_gb[:], ones1b[:], gb_rowb[:])
            gb_bc = cpool.tile([128, 2 * D], F32)
            nc.vector.tensor_copy(gb_bc[:], p_gb[:])
            g_bc = gb_bc[:, 0:D]
            b_bc = gb_bc[:, D:2 * D]

            # ================= LN stats, batched [128, 5] ==================
            # col 0: U quadrant tile; cols 1+2e+n: example e slot n
            sum5 = pool.tile([128, 5], F32, tag="sum5")
            nc.vector.reduce_sum(sum5[:, 1:3], in_tiles[0][:], axis=AX.X)
            nc.vector.reduce_sum(sum5[:, 0:1], tU4[:], axis=AX.X)
            nc.vector.reduce_sum(sum5[:, 3:5], in_tiles[1][:], axis=AX.X)
            nm5 = pool.tile([128, 5], F32, tag="nm5")
            nc.vector.tensor_scalar_mul(nm5[:], sum5[:], -1.0 / D)
            xc5 = pool.tile([128, 5, D], F32, tag="xc5")
            for e in range(B_LOC):
                for n in range(2):
                    k = 1 + 2 * e + n
                    nc.vector.tensor_scalar_add(xc5[:, k, :], in_tiles[e][:, n, :],
                                                nm5[:, k:k + 1])
            nc.vector.tensor_scalar_add(xc5[:, 0, :], tU4[:], nm5[:, 0:1])
            sq5 = pool.tile([128, D], F32, tag="sq5")
            ss5 = pool.tile([128, 5], F32, tag="ss5")
            for k in [1, 2, 3, 4, 0]:
                nc.scalar.activation(sq5[:], xc5[:, k, :], ACTF.Square,
                                     accum_out=ss5[:, k:k + 1])
            xv5 = pool.tile([128, 5], F32, tag="xv5")
            nc.vector.tensor_scalar(xv5[:], ss5[:], 1.0 / D, EPS, op0=ALU.mult, op1=ALU.add)
            rstd5 = _rsqrt(nc, pool, xv5, 128, 5, "ln5")

            # U rows LN (full tile; only quadrant rows are real)
            elnU4 = cpool.tile([128, D], F32)
            nc.vector.scalar_tensor_tensor(elnU4[:], xc5[:, 0, :], rstd5[:, 0:1],
                                           g_bc, op0=ALU.mult, op1=ALU.mult)
            nc.vector.tensor_add(elnU4[:], elnU4[:], b_bc)

            st = [dict() for _ in range(B_LOC)]

            # ============ per-example u0/iid rows + broadcast ==============
            for e in range(B_LOC):
                S = st[e]
                iid_row = pool.tile([1, D], F32, tag=f"iid{e}")
                nc.sync.dma_start(iid_row[:], elnU4[64 * e + 32:64 * e + 33, :])
                S["iid_row"] = iid_row
                if e == 0:
                    u0row = elnU4[0:1, :]
                else:
                    u0r = pool.tile([1, D], F32, tag="u0r")
                    nc.scalar.copy(u0r[:], elnU4[64:65, :])
                    u0row = u0r[:]
                S["u0row"] = u0row
                # [u0*g | u0*b] row, broadcast to all partitions via PE
                ugb_row = pool.tile([1, 2 * D], BF16, tag=f"ugb{e}")
                nc.vector.tensor_mul(ugb_row[:, 0:D], u0row, g_row)
                nc.vector.tensor_mul(ugb_row[:, D:2 * D], u0row, b_row)
                p_ugb = pmid.tile([128, 2 * D], F32, tag="mid")
                nc.tensor.matmul(p_ugb[:], ones1b[:], ugb_row[:])
                S["p_ugb"] = p_ugb

                # c = vi . iid + c0, staged at partition 0 of a [1,1] tile
                si_scr = pool.tile([1, D], F32, tag="si_scr")
                nc.vector.tensor_mul(si_scr[:], iid_row[:], vi_row)
                si = pool.tile([1, 1], F32, tag="si")
                nc.vector.reduce_sum(si[:], si_scr[:], axis=AX.X)
                c_all = pool.tile([1, 1], F32, tag="c_all")
                nc.vector.tensor_scalar_add(c_all[:], si[:], c0_sb[:])
                S["c_all"] = c_all

            # ============ per-example ua (bf16) + transposes ===============
            for e in range(B_LOC):
                S = st[e]
                p_ugb = S["p_ugb"]
                ua_bf = pool.tile([128, 2, D], BF16, tag=f"ua{e}")
                for n in range(2):
                    k = 1 + 2 * e + n
                    t = pool.tile([128, D], F32, tag=f"uat{n}")
                    nc.vector.scalar_tensor_tensor(t[:], xc5[:, k, :], rstd5[:, k:k + 1],
                                                   p_ugb[:, 0:D], op0=ALU.mult, op1=ALU.mult)
                    nc.vector.tensor_add(ua_bf[:, n, :], t[:], p_ugb[:, D:2 * D])
                S["ua_bf"] = ua_bf

                p_t = pmid.tile([D, N], BF16, tag="mid")
                nc.tensor.transpose(p_t[:, 0:128], ua_bf[:, 0, :], identB[:])
                nc.tensor.transpose(p_t[:, 128:256], ua_bf[:, 1, :], identB[:])
                uat = pool.tile([D, N], BF16, tag=f"uatT{e}")
                nc.vector.tensor_copy(uat[:], p_t[:])
                ua2t = pool.tile([D, N], BF16, tag=f"ua2t{e}")
                nc.scalar.activation(ua2t[:], p_t[:], ACTF.Square)
                S["uat"] = uat
                S["ua2t"] = ua2t

            # ============ per-example scores ===============================
            for e in range(B_LOC):
                S = st[e]
                uat = S["uat"]
                # s_k as columns (psum partition = j), s_q as a row; one psum
                # tile per example: cols 0:256 = s_q row, 256:260 = s_k cols
                skc = pool.tile([128, 2], F32, tag=f"skc{e}")
                p_sc = psmall.tile([128, N + 4], F32, tag="small")
                for J in range(2):
                    cs = slice(J * 128, (J + 1) * 128)
                    nc.tensor.matmul(p_sc[:, N + 2 * J:N + 2 * J + 2], uat[:, cs], vqk_bf[:])
                    nc.vector.tensor_copy(skc[:, J:J + 1], p_sc[:, N + 2 * J + 1:N + 2 * J + 2])
                nc.tensor.matmul(p_sc[0:1, 0:N], vqk_bf[:, 0:1], uat[:])
                sqc = pool.tile([1, N], BF16, tag=f"sqc{e}")
                nc.scalar.activation(sqc[:], p_sc[0:1, 0:N], ACTF.Identity,
                                     bias=S["c_all"][:])
                S["skc"] = skc
                S["sqc"] = sqc

            # ============ qk^T -> merged exp ===============================
            # qkT_big free layout: [e0J0 | e0J1 | e1J0 | e1J1], 256 cols each
            qkT_big = pool.tile([128, 4, N], F32, tag="qkT")
            for e in range(B_LOC):
                S = st[e]
                p_sqbc = pqk.tile([128, N], F32, tag="qk")
                nc.tensor.matmul(p_sqbc[:], ones1b[:], S["sqc"][:])
                for J in range(2):
                    nc.scalar.activation(qkT_big[:, 2 * e + J, :], p_sqbc[:], ACTF.Prelu,
                                         bias=S["skc"][:, J:J + 1], alpha=SLOPE)
            exp_big = pool.tile([128, 4, N], BF16, tag="expb")
            ei = nc.scalar.activation(exp_big[:].rearrange("p a b -> p (a b)"),
                                      qkT_big[:].rearrange("p a b -> p (a b)"), ACTF.Exp)

            # ============ grams -> merged var -> isig ======================
            p_mu = pgram.tile([128, 4, N], F32, tag="gram")
            p_e2 = pgram.tile([128, 4, N], F32, tag="gram")
            for e in range(B_LOC):
                S = st[e]
                for blk in range(2):
                    cs = slice(blk * 128, (blk + 1) * 128)
                    nc.tensor.matmul(p_mu[:, 2 * e + blk, :], S["uat"][:, cs], S["uat"][:])
                    nc.tensor.matmul(p_e2[:, 2 * e + blk, :], S["ua2t"][:, cs], S["ua2t"][:])
            msq = pool.tile([128, 4 * N], F32, tag="msq")
            nc.scalar.activation(msq[:], p_mu[:].rearrange("p a b -> p (a b)"),
                                 ACTF.Square, scale=1.0 / D)
            var = pool.tile([128, 4 * N], F32, tag="var")
            nc.vector.scalar_tensor_tensor(var[:], p_e2[:].rearrange("p a b -> p (a b)"),
                                           1.0 / D, msq[:], op0=ALU.mult, op1=ALU.subtract)
            isig = pool.tile([128, 4, N], BF16, tag="isig")
            ri = nc.scalar.activation(isig[:].rearrange("p a b -> p (a b)"), var[:],
                                      ACTF.Abs_reciprocal_sqrt)
            add_dep_helper(ri.ins, ei.ins, sync=False,
                           reason="abs-rsqrt after the exp-set ACT ops")

            # merged beta~^T = exp * isig (layouts match: J == blk by symmetry)
            btT_big = pool.tile([128, 4, N], BF16, tag="btT")
            nc.vector.tensor_mul(btT_big[:].rearrange("p a b -> p (a b)"),
                                 exp_big[:].rearrange("p a b -> p (a b)"),
                                 isig[:].rearrange("p a b -> p (a b)"))

            # ============ denominators =====================================
            for e in range(B_LOC):
                S = st[e]
                rden_cols = pool.tile([128, 2], F32, tag=f"rdenc{e}")
                p_den = psmall.tile([128, 4], F32, tag="small")
                for blk in range(2):
                    cs = slice(blk * 128, (blk + 1) * 128)
                    ds = slice(2 * blk, 2 * blk + 2)
                    nc.tensor.matmul(p_den[:, ds], exp_big[:, 2 * e, cs], ones2b[:],
                                     start=True, stop=False)
                    nc.tensor.matmul(p_den[:, ds], exp_big[:, 2 * e + 1, cs], ones2b[:],
                                     start=False, stop=True)
                    nc.vector.reciprocal(rden_cols[:, blk:blk + 1], p_den[:, 2 * blk:2 * blk + 1])
                S["rden_cols"] = rden_cols

            # ================= pass B: attention + output ==================
            for e in range(B_LOC):
                S = st[e]
                ua_bf = S["ua_bf"]

                p_S2 = pmid.tile([128, 2, D], F32, tag="mid")
                for blk in range(2):
                    cs = slice(blk * 128, (blk + 1) * 128)
                    nc.tensor.matmul(p_S2[:, blk, :], btT_big[:, 2 * e, cs], ua_bf[:, 0, :],
                                     start=True, stop=False)
                    nc.tensor.matmul(p_S2[:, blk, :], btT_big[:, 2 * e + 1, cs], ua_bf[:, 1, :],
                                     start=False, stop=True)

                t1b = pool.tile([128, 2, D], F32, tag="t1b")
                nc.vector.tensor_mul(t1b[:], ua_bf[:], p_S2[:])
                c_raw = pool.tile([128, 2], F32, tag="c_raw")
                nc.vector.reduce_sum(c_raw[:], t1b[:], axis=AX.X)
                c_col = pool.tile([128, 2], F32, tag="c_col")
                nc.vector.tensor_scalar_mul(c_col[:], c_raw[:], 1.0 / D)

                o_big = pool.tile([128, 2, D], F32, tag="o_big")
                for blk in range(2):
                    rg = pool.tile([128, D], F32, tag="rg")
                    nc.scalar.activation(rg[:], g_bc, ACTF.Copy,
                                         scale=S["rden_cols"][:, blk:blk + 1])
                    t2 = pool.tile([128, D], F32, tag="t2")
                    nc.vector.scalar_tensor_tensor(t2[:], t1b[:, blk, :], c_col[:, blk:blk + 1],
                                                   rg[:], op0=ALU.subtract, op1=ALU.mult)
                    t3 = pool.tile([128, D], F32, tag="t3")
                    nc.vector.tensor_add(t3[:], t2[:], b_bc)
                    nc.scalar.activation(o_big[:, blk, :], t3[:], ACTF.Prelu, alpha=SLOPE)
                out_rows = out[e, 1:257, :].rearrange("(p n) d -> p n d", n=2)
                (nc.sync if e == 0 else nc.gpsimd).dma_start(out_rows, o_big[:])

                ui = pool.tile([1, D], F32, tag="ui")
                nc.vector.tensor_mul(ui[:], S["u0row"], S["iid_row"][:])
                uo = pool.tile([1, D], F32, tag="uo")
                nc.vector.scalar_tensor_tensor(uo[:], ui[:], SLOPE, ui[:],
                                               op0=ALU.mult, op1=ALU.max)
                (nc.sync if e == 0 else nc.gpsimd).dma_start(out[e, 0:1, :], uo[:])

    nc.compile()
    return nc


def _host_consts(Wa, ba, a_w, a_b):
    aq, ak, ai = a_w[:D], a_w[D:2 * D], a_w[2 * D:]
    vq = aq @ Wa
    vk = ak @ Wa
    vi = ai @ Wa
    c0 = float(ba @ aq + ba @ ak + ba @ ai + a_b[0])
    cstT = np.stack([vq, vk], axis=1).astype(np.float32)
    cstR = np.zeros((1, 4 * D), np.float32)
    cstR[0, 2 * D:3 * D] = vi
    cstR[0, 3 * D] = c0
    return cstT, cstR


_NC_CACHE = {}


def _get_nc():
    if "nc" not in _NC_CACHE:
        _NC_CACHE["nc"] = build()
    return _NC_CACHE["nc"]


def run(embeddings, Wa, ba, a_w, a_b, ln_g, ln_b, **spmd_kwargs):
    embeddings = np.ascontiguousarray(embeddings, dtype=np.float32)
    cstT, cstR = _host_consts(np.asarray(Wa, np.float32), np.asarray(ba, np.float32),
                              np.asarray(a_w, np.float32), np.asarray(a_b, np.float32))
    cstR[0, 0:D] = np.asarray(ln_g, np.float32)
    cstR[0, D:2 * D] = np.asarray(ln_b, np.float32)

    nc = _get_nc()
    in_maps = [
        {"emb": embeddings[c * B_LOC:(c + 1) * B_LOC], "cstT": cstT, "cstR": cstR}
        for c in range(N_CORES)
    ]
    res = run_bass_kernel_spmd(nc, in_maps, core_ids=list(range(N_CORES)), **spmd_kwargs)
    outp = np.concatenate([res.results[c]["out"] for c in range(N_CORES)], axis=0)
    return outp, res


def kernel(embeddings, Wa, ba, a_w, a_b, ln_g, ln_b):
    outp, _ = run(embeddings, Wa, ba, a_w, a_b, ln_g, ln_b)
    return outp
